# revision 15
# baseline (speedup 1.0000x reference)
"""Causal self-attention (B=4, T=2048, C=1024, H=16) on 8 Trainium2 NeuronCores.

Sharding: tensor-parallel over heads. Each core owns 2 heads:
  - Wq/Wk/Wv column slices [C, 128], Wo row slice [128, C]
  - computes q/k/v for its heads from the full x, flash-style causal
    attention, and a partial output projection.
  - host sums the 8 partial outputs and adds bo.

Device algorithm per (batch b):
  xT[b] (pre-transposed on host to [C, T]) is DMA'd in 128-row tiles.
  qT/kT/vT [128(2 heads x 64d), T] = W.T @ xT  (+bias via ACT Identity)
  v_aug [128 t, 65] per head via PE transpose of vT (ones col appended).
  Scores (transposed): ST[j-tile 128, i-chunk 512] = kT.T @ qT per head,
    both heads packed with row-tiled matmuls (K=64 at rows 0 / 64).
  E = exp(ST) on ACT (1/sqrt(D) folded into Wq on host; no max-subtract
    needed: scores are O(5) so exp is safe in fp32).
  Causal mask on diagonal blocks via gpsimd affine_select (fill 0).
  yT_aug[65, i-chunk] += v_aug.T @ E accumulated over j in PSUM; row 64
    is the softmax denominator (ones column trick).
  alpha = 1/denominator (DVE reciprocal), broadcast across partitions
    (gpsimd partition_broadcast), yTa = yT * alpha (DVE, PSUM->SBUF).
  out_partial[i-tile 128, c-chunk 512] = sum_h yTa_h.T @ Wo_h, DMA'd out.

All matmuls run in float32r (full PE rate at free-dim >= 256).
"""

import sys

if "/opt/trn_rl_repo" not in sys.path:
    sys.path.insert(0, "/opt/trn_rl_repo")

from contextlib import ExitStack

import numpy as np

import concourse.bass as bass
import concourse.tile as tile
from concourse import bacc, mybir
from concourse import bass_utils
from concourse import masks

B, T, C, H, D = 4, 2048, 1024, 16, 64
N_CORES = 8
HPC = H // N_CORES  # heads per core = 2
W = HPC * D  # per-core projection width = 128

F32 = mybir.dt.float32
F32R = mybir.dt.float32r
F16 = mybir.dt.float16
AF = mybir.ActivationFunctionType

ICH = 512  # i (query) chunk in the free dim
LOG16 = float(np.log(16.0))
NIC = T // ICH  # 4
NKT = C // 128  # 8 contraction tiles for projections
NJT = T // 128  # 16 key tiles

_CACHE = {}


def _kernel_body(ctx, tc, xT, wq, wk, wv, wo0, wo1, bq, bk, bv, ones, ones16r, out):
    nc = tc.nc

    const_p = ctx.enter_context(tc.tile_pool(name="const", bufs=1))
    w_p = ctx.enter_context(tc.tile_pool(name="wts", bufs=1))
    xt_p = ctx.enter_context(tc.tile_pool(name="xt", bufs=NKT))
    act_p = ctx.enter_context(tc.tile_pool(name="acts", bufs=2))
    va_p = ctx.enter_context(tc.tile_pool(name="vaug", bufs=20))
    e_p = ctx.enter_context(tc.tile_pool(name="ep", bufs=5))
    yta_p = ctx.enter_context(tc.tile_pool(name="yta", bufs=8))
    r_p = ctx.enter_context(tc.tile_pool(name="alpha", bufs=4))
    ob_p = ctx.enter_context(tc.tile_pool(name="ob", bufs=4))
    pmisc = ctx.enter_context(tc.tile_pool(name="pmisc", bufs=3, space="PSUM"))
    ps_p = ctx.enter_context(tc.tile_pool(name="psc", bufs=3, space="PSUM"))
    py_p = ctx.enter_context(tc.tile_pool(name="py", bufs=2, space="PSUM"))

    # constants / weights (loaded once)
    ident = const_p.tile([128, 128], F16, tag="ident")
    masks.make_identity(nc, ident[:])

    bias_q = const_p.tile([W, 1], F32, tag="bq")
    bias_k = const_p.tile([W, 1], F32, tag="bk")
    bias_v = const_p.tile([W, 1], F32, tag="bv")
    ones_sb = const_p.tile([128, 64], F32, tag="ones")
    nc.sync.dma_start(ones_sb[:], ones[:])
    ones16 = const_p.tile([128, 64], F16, tag="ones16")
    nc.sync.dma_start(ones16[:], ones16r[:])
    log16 = const_p.tile([128, 1], F32, tag="log16")
    nc.gpsimd.memset(log16[:], LOG16)
    nc.sync.dma_start(bias_q[:], bq[:])
    nc.sync.dma_start(bias_k[:], bk[:])
    nc.sync.dma_start(bias_v[:], bv[:])

    # projection weights: [128 c-part, 128 d] per k-tile, packed along free dim
    wq_sb = w_p.tile([128, C], F16, tag="wq")
    wk_sb = w_p.tile([128, C], F16, tag="wk")
    wv_sb = w_p.tile([128, C], F16, tag="wv")
    for kt in range(NKT):
        sl = slice(kt * 128, (kt + 1) * 128)
        nc.sync.dma_start(wq_sb[:, sl], wq[sl, :])
        nc.sync.dma_start(wk_sb[:, sl], wk[sl, :])
        nc.sync.dma_start(wv_sb[:, sl], wv[sl, :])
    wo0_sb = w_p.tile([D, C], F16, tag="wo0")
    wo1_sb = w_p.tile([D, C], F16, tag="wo1")
    nc.sync.dma_start(wo0_sb[:], wo0[:])
    nc.sync.dma_start(wo1_sb[:], wo1[:])

    pending_outproj = []
    for b in range(B):
        # ---- load xT[b] ----
        xts = []
        for kt in range(NKT):
            xt = xt_p.tile([128, T], F16, tag="xt")
            nc.sync.dma_start(xt[:], xT[b, kt * 128 : (kt + 1) * 128, :])
            xts.append(xt)

        # ---- projections: qT/kT/vT [128, T] ----
        qT = act_p.tile([128, T], F16, tag="qT")
        kT = act_p.tile([128, T], F16, tag="kT")
        vT = act_p.tile([128, T], F16, tag="vT")
        for n in range(NIC):
            csl = slice(n * ICH, (n + 1) * ICH)
            psq = pmisc.tile([128, ICH], F32, tag="pp")
            psk = pmisc.tile([128, ICH], F32, tag="pp")
            psv = pmisc.tile([128, ICH], F32, tag="pp")
            for kt in range(NKT):
                wsl = slice(kt * 128, (kt + 1) * 128)
                st, sp = kt == 0, kt == NKT - 1
                nc.tensor.matmul(psq[:], wq_sb[:, wsl], xts[kt][:, csl], start=st, stop=sp)
                nc.tensor.matmul(psk[:], wk_sb[:, wsl], xts[kt][:, csl], start=st, stop=sp)
                nc.tensor.matmul(psv[:], wv_sb[:, wsl], xts[kt][:, csl], start=st, stop=sp)
            nc.vector.tensor_scalar_add(qT[:, csl], psq[:], bias_q[:])
            nc.vector.tensor_scalar_add(kT[:, csl], psk[:], bias_k[:])
            nc.vector.tensor_scalar_add(vT[:, csl], psv[:], bias_v[:])

        # ---- v_aug tiles: [128 t, 130] = [h0 d64 | ones | h1 d64 | ones] ----
        vas = []
        for tt in range(NJT):
            if tt == 4 and pending_outproj:
                pending_outproj.pop(0)()
            pst = ps_p.tile([128, 128], F16, tag="ps")
            nc.tensor.transpose(pst[:], vT[:, tt * 128 : (tt + 1) * 128], ident[:])
            va = va_p.tile([128, 130], F16, tag="va")
            nc.vector.tensor_copy(va[:, 0:64], pst[:, 0:64])
            nc.vector.tensor_copy(va[:, 65:129], pst[:, 64:128])
            nc.vector.tensor_copy(va[:, 64:65], ones16[:, 0:1])
            nc.vector.tensor_copy(va[:, 129:130], ones16[:, 0:1])
            vas.append(va)

        # ---- attention + normalization per i-chunk ----
        ytas = []  # [(yta_h0, yta_h1)] per ic, each [64, ICH] fp16
        for ic in range(NIC):
            i0 = ic * ICH
            isl = slice(i0, i0 + ICH)
            njt = (i0 + ICH) // 128
            py0 = py_p.tile([65, ICH], F32, tag="py")
            py1 = py_p.tile([65, ICH], F32, tag="py")
            for jt in range(njt):
                if jt == 2 and pending_outproj:
                    pending_outproj.pop(0)()
                j0 = jt * 128
                jsl = slice(j0, j0 + 128)
                ps0 = ps_p.tile([128, ICH], F32, tag="ps")
                ps1 = ps_p.tile([128, ICH], F32, tag="ps")
                nc.tensor.matmul(
                    ps0[:], kT[0:64, jsl], qT[0:64, isl],
                    start=True, stop=True, tile_position=(0, 0),
                )
                nc.tensor.matmul(
                    ps1[:], kT[64:128, jsl], qT[64:128, isl],
                    start=True, stop=True, tile_position=(64, 0),
                )
                e0 = e_p.tile([128, ICH], F16, tag="e")
                e1 = e_p.tile([128, ICH], F16, tag="e")
                nc.scalar.activation(e0[:], ps0[:], AF.Exp)
                nc.scalar.activation(e1[:], ps1[:], AF.Exp)
                if j0 + 127 > i0:  # diagonal block: zero where j > i
                    for e in (e0, e1):
                        nc.gpsimd.affine_select(
                            out=e[:], in_=e[:],
                            pattern=[[1, ICH]],
                            compare_op=mybir.AluOpType.is_ge,
                            fill=0.0,
                            base=i0 - j0,
                            channel_multiplier=-1,
                        )
                st, sp = jt == 0, jt == njt - 1
                nc.tensor.matmul(py0[:], vas[jt][:, 0:65], e0[:], start=st, stop=sp)
                nc.tensor.matmul(py1[:], vas[jt][:, 65:130], e1[:], start=st, stop=sp)

            pair = []
            lnts = []
            for py in (py0, py1):
                lnt = r_p.tile([65, ICH], F32, tag="lnt")
                nc.scalar.activation(lnt[64:65, :], py[64:65, :], AF.Ln)
                lnts.append(lnt)
            for py, lnt in zip((py0, py1), lnts):
                # r = 16/denom (x16 keeps r in fp16-normal range; /16 folded into Wo)
                r = r_p.tile([65, ICH], F16, tag="r")
                nc.scalar.activation(r[64:65, :], lnt[64:65, :], AF.Exp, scale=-1.0, bias=log16[64:65, :])
                ab = ps_p.tile([64, ICH], F32, tag="ps")
                nc.tensor.matmul(
                    ab[:], ones16[64:65, :], r[64:65, :],
                    start=True, stop=True, tile_position=(64, 0),
                )
                ab_sb = r_p.tile([64, ICH], F32, tag="absb")
                nc.vector.tensor_copy(ab_sb[:], ab[:])
                yta = yta_p.tile([64, ICH], F16, tag="yta")
                nc.vector.tensor_mul(yta[:], py[0:64, :], ab_sb[:])
                pair.append(yta)
            ytas.append(pair)

            # out-projection for this i-chunk: queued, emitted one phase later
            # so the in-order PE stream has next-chunk scores to run while the
            # softmax chain (Ln/Exp/bcast/mult) completes.
            def _outproj(b=b, ic=ic, y0=pair[0], y1=pair[1]):
                for itl in range(ICH // 128):
                    off = itl * 128
                    it = ic * 4 + itl
                    for nch in range(C // ICH):
                        osl = slice(nch * ICH, (nch + 1) * ICH)
                        po = pmisc.tile([128, ICH], F32, tag="pp")
                        nc.tensor.matmul(
                            po[:], y0[:, off : off + 128], wo0_sb[:, osl], start=True, stop=False
                        )
                        nc.tensor.matmul(
                            po[:], y1[:, off : off + 128], wo1_sb[:, osl], start=False, stop=True
                        )
                        ob = ob_p.tile([128, ICH], F32, tag="ob")
                        nc.vector.tensor_copy(ob[:], po[:])
                        nc.sync.dma_start(out[b, it * 128 : (it + 1) * 128, osl], ob[:])
            pending_outproj.append(_outproj)

    # flush remaining queued out-projections
    while pending_outproj:
        pending_outproj.pop(0)()


def _build():
    if "nc" in _CACHE:
        return _CACHE["nc"]
    nc = bacc.Bacc("TRN2", target_bir_lowering=False, debug=False, num_devices=N_CORES)
    xT = nc.dram_tensor("xT", [B, C, T], F16, kind="ExternalInput").ap()
    wq = nc.dram_tensor("wq", [C, W], F16, kind="ExternalInput").ap()
    wk = nc.dram_tensor("wk", [C, W], F16, kind="ExternalInput").ap()
    wv = nc.dram_tensor("wv", [C, W], F16, kind="ExternalInput").ap()
    wo0 = nc.dram_tensor("wo0", [D, C], F16, kind="ExternalInput").ap()
    wo1 = nc.dram_tensor("wo1", [D, C], F16, kind="ExternalInput").ap()
    bq = nc.dram_tensor("bq", [W, 1], F32, kind="ExternalInput").ap()
    bk = nc.dram_tensor("bk", [W, 1], F32, kind="ExternalInput").ap()
    bv = nc.dram_tensor("bv", [W, 1], F32, kind="ExternalInput").ap()
    ones = nc.dram_tensor("ones", [128, 64], F32, kind="ExternalInput").ap()
    ones16r = nc.dram_tensor("ones16r", [128, 64], F16, kind="ExternalInput").ap()
    out = nc.dram_tensor("out", [B, T, C], F32, kind="ExternalOutput").ap()

    with tile.TileContext(nc) as tc:
        with ExitStack() as ctx:
            _kernel_body(ctx, tc, xT, wq, wk, wv, wo0, wo1, bq, bk, bv, ones, ones16r, out)
    nc.compile()
    _CACHE["nc"] = nc
    return nc


def make_in_maps(inputs):
    x = np.asarray(inputs["x"], np.float32)
    Wq = np.asarray(inputs["Wq"], np.float32)
    bq = np.asarray(inputs["bq"], np.float32)
    Wk = np.asarray(inputs["Wk"], np.float32)
    bk = np.asarray(inputs["bk"], np.float32)
    Wv = np.asarray(inputs["Wv"], np.float32)
    bv = np.asarray(inputs["bv"], np.float32)
    Wo = np.asarray(inputs["Wo"], np.float32)

    scale = np.float32(1.0 / np.sqrt(D))
    xT = np.ascontiguousarray(x.transpose(0, 2, 1))  # [B, C, T]
    Wq_s = Wq * scale
    bq_s = bq * scale

    in_maps = []
    for c in range(N_CORES):
        s = slice(c * W, (c + 1) * W)
        in_maps.append(
            {
                "xT": xT.astype(np.float16),
                "wq": np.ascontiguousarray(Wq_s[:, s]).astype(np.float16),
                "wk": np.ascontiguousarray(Wk[:, s]).astype(np.float16),
                "wv": np.ascontiguousarray(Wv[:, s]).astype(np.float16),
                "wo0": np.ascontiguousarray(Wo[c * W : c * W + D, :] / 16.0).astype(np.float16),
                "wo1": np.ascontiguousarray(Wo[c * W + D : (c + 1) * W, :] / 16.0).astype(np.float16),
                "bq": np.ascontiguousarray(bq_s[s, None]),
                "bk": np.ascontiguousarray(bk[s, None]),
                "bv": np.ascontiguousarray(bv[s, None]),
                "ones": np.ones((128, 64), np.float32),
                "ones16r": np.ones((128, 64), np.float16),
            }
        )
    return in_maps


def kernel(**inputs):
    nc = _build()
    in_maps = make_in_maps(inputs)
    res = bass_utils.run_bass_kernel_spmd(nc, in_maps, core_ids=list(range(N_CORES)))
    bo = np.asarray(inputs["bo"], np.float32)
    out = np.zeros((B, T, C), np.float32)
    for c in range(N_CORES):
        out += res.results[c]["out"]
    out += bo
    return out


if __name__ == "__main__":
    rng = np.random.default_rng(0)
    ins = {
        "x": rng.standard_normal((B, T, C), dtype=np.float32),
        "Wq": rng.standard_normal((C, C), dtype=np.float32) / 32,
        "bq": rng.standard_normal((C,), dtype=np.float32) * 0.02,
        "Wk": rng.standard_normal((C, C), dtype=np.float32) / 32,
        "bk": rng.standard_normal((C,), dtype=np.float32) * 0.02,
        "Wv": rng.standard_normal((C, C), dtype=np.float32) / 32,
        "bv": rng.standard_normal((C,), dtype=np.float32) * 0.02,
        "Wo": rng.standard_normal((C, C), dtype=np.float32) / 32,
        "bo": rng.standard_normal((C,), dtype=np.float32) * 0.02,
    }
    got = kernel(**ins)
    print("kernel ran, out shape", got.shape)
